# revision 4
# baseline (speedup 1.0000x reference)
"""Trainium2 Bass kernel for ExampleGNN (2-layer GCN + global_add_pool + head).

Self-contained: accepts FULL inputs, shards across 8 NeuronCores internally,
returns the FULL [64, 32] log-softmax output.

Sharding: nodes (and their incident in-edges) are partitioned across 8 cores
with a degree-balancing permutation (node relabeling is internal; pooling is
order-invariant). Weights replicated. One AllGather shares layer-1
activations between layers; one AllReduce combines pooled partials.

v3 pipeline (per core, per layer):
  - norm factored: gather-table rows pre-scaled by dinv[src] on host;
    dinv[dst] applied via the post-matmul activation scale
  - tables stored as [N/2, 256] bf16 "pair" rows; a parity-p gather uses a
    column-sliced AP (stride 2 rows) so idx = src//2 fits int16 with a
    single table; tiles are parity-pure
  - BLK=128 dst nodes per aggregation block (block == h-chunk)
  - gather calls decoupled from blocks: runs of parity-pure tiles spanning
    a group of GBLK blocks, one dma_gather + one B-matrix DMA per call
  - PE accumulates aggT[f, n] += gathered^T @ B (fp8 one-hot B) into PSUM
    per block; self-loop joins via a diagonal matmul from a resident slab
  - h chunks: PSUM h = aggT^T W (bf16, FWL), ACT relu with per-node scale
    written straight into the layer-2 slab (layer 1) or pooled (layer 2)
"""
import numpy as np

import concourse.bacc as bacc
import concourse.mybir as mybir
import concourse.tile as tile

CORES = 8
N = 50000
D = 128
DOUT = 32
G = 64
NPC = N // CORES           # 6250 nodes per core
NPAIR = N // 2             # pair rows in gather tables
BLK = 128                  # aggregation block (= h chunk)
NBLK = (NPC + BLK - 1) // BLK   # 49 blocks (last has 106 nodes)
GBLK = 4                   # blocks per gather group (parity-pure runs)
CT_MAX = 32                # max tiles (128 idxs each) per dma_gather call
QUEUES = 4                 # SWDGE queues (ucode max)
GATHER_BUFS = 6
B_BUFS = 6
AGG_BUFS = 4
HPS_BUFS = 2

f32 = mybir.dt.float32
bf16 = mybir.dt.bfloat16
fp8 = mybir.dt.float8e4   # one-hot dtype (values 0/1 exact)
i16 = mybir.dt.int16


# ---------------------------------------------------------------- host prep --

def _np_dt(dt):
    return mybir.dt.np(dt)


def _wrap_idxs(idx):
    """[n] -> [128, n//16] int16 wrapped layout (16-partition groups,
    replicated for the 8 gpsimd cores)."""
    n = len(idx)
    t = np.asarray(idx, dtype=np.int16).reshape(n // 16, 16).T
    return np.ascontiguousarray(np.tile(t, (8, 1)))


def prep(edge_index, batch):
    """Host-side index prep. Returns (structure, per_core arrays, node perm,
    dinv in old-id space)."""
    src_o = np.asarray(edge_index[0], dtype=np.int64)
    dst_o = np.asarray(edge_index[1], dtype=np.int64)
    deg = (np.bincount(dst_o, minlength=N) + 1).astype(np.float32)
    dinv = (1.0 / np.sqrt(deg)).astype(np.float32)

    # ---- LPT balance: assign nodes (by desc in-degree) to 8*NBLK bins
    nbins = CORES * NBLK
    cap = np.full(nbins, BLK, dtype=np.int64)
    cap[NBLK - 1::NBLK] = NPC - (NBLK - 1) * BLK   # last block per core: 106
    order = np.argsort(-deg, kind="stable")
    fill = np.zeros(nbins, dtype=np.int64)
    perm = np.empty(N, dtype=np.int64)
    import heapq
    heap = [(0.0, int(b)) for b in range(nbins)]
    heapq.heapify(heap)
    for nid in order:
        while True:
            l, b = heapq.heappop(heap)
            if fill[b] < cap[b]:
                break
        c, blk_i = divmod(b, NBLK)
        perm[nid] = c * NPC + blk_i * BLK + fill[b]
        fill[b] += 1
        l += float(deg[nid])
        if fill[b] < cap[b]:
            heapq.heappush(heap, (l, b))

    src = perm[src_o]
    dst = perm[dst_o]

    core = dst // NPC
    dstloc = dst - core * NPC
    blk = dstloc // BLK
    dsub = (dstloc % BLK).astype(np.int64)
    par = (src % 2).astype(np.int64)
    idx16 = (src // 2).astype(np.int16)

    so = np.lexsort((par, blk, core))
    idx_s, dsub_s = idx16[so], dsub[so]
    core_s, blk_s, par_s = core[so], blk[so], par[so]

    cnt = np.zeros((CORES, NBLK, 2), dtype=np.int64)
    np.add.at(cnt, (core_s, blk_s, par_s), 1)
    ccap = -(-cnt.max(axis=0) // 128) * 128     # [NBLK, 2] rows, 128-aligned

    # ---- tile sequence: per GBLK-group, parity-0 run then parity-1 run
    tiles = []        # (b, p) per 128-idx tile
    calls = []        # (t0, ct, p)
    seg_order = []    # (b, p, ntiles) in tile order, for idx/bmat layout
    for g0 in range(0, NBLK, GBLK):
        for p in (0, 1):
            run_t0 = len(tiles)
            for b in range(g0, min(g0 + GBLK, NBLK)):
                ntb = int(ccap[b, p]) // 128
                if ntb:
                    seg_order.append((b, p, ntb))
                    tiles += [(b, p)] * ntb
            t = run_t0
            while t < len(tiles):
                ct = min(CT_MAX, len(tiles) - t)
                calls.append((t, ct, p))
                t += ct
    ttot = len(tiles)
    itot = ttot * 128
    nt_block = [0] * NBLK
    for (b, p) in tiles:
        nt_block[b] += 1

    starts = np.cumsum(np.concatenate([[0], cnt.reshape(-1)]))[:-1].reshape(cnt.shape)
    fp8np = _np_dt(fp8)
    per_core = []
    for c in range(CORES):
        idx_flat = np.zeros(itot, dtype=np.int16)
        bcol = np.full(itot, -1, dtype=np.int64)  # -1 = pad row
        pos = 0
        for (b, p, ntb) in seg_order:
            take = int(cnt[c, b, p])
            if take > 0:
                sl = slice(int(starts[c, b, p]), int(starts[c, b, p]) + take)
                idx_flat[pos:pos + take] = idx_s[sl]
                bcol[pos:pos + take] = dsub_s[sl]
            pos += ntb * 128
        # B one-hot: rows laid [tile, row-in-tile]; partition = row-in-tile
        bm = np.zeros((itot, BLK), dtype=fp8np)
        rr = np.nonzero(bcol >= 0)[0]
        bm[rr, bcol[rr]] = 1.0
        bm = bm.reshape(ttot, 128, BLK).transpose(1, 0, 2).reshape(128, ttot * BLK)
        per_core.append({
            "idx": _wrap_idxs(idx_flat),
            "bmat": np.ascontiguousarray(bm),
        })

    # per new-node-id vectors
    batch = np.asarray(batch, dtype=np.int64)
    batch_new = np.zeros(N, dtype=np.int64)
    batch_new[perm] = batch
    dinv_new = np.zeros(N, dtype=np.float32)
    dinv_new[perm] = dinv
    for c in range(CORES):
        lo, hi = c * NPC, (c + 1) * NPC
        dv = np.zeros(NBLK * 128, dtype=np.float32)
        dv[:NPC] = dinv_new[lo:hi]
        per_core[c]["dinvc"] = np.ascontiguousarray(dv.reshape(NBLK, 128).T)
        per_core[c]["dinv2c"] = np.ascontiguousarray(
            (dv * dv).reshape(NBLK, 128).T)
        rd = np.ones(NBLK * 128, dtype=np.float32)
        rd[:NPC] = 1.0 / dinv_new[lo:hi]
        per_core[c]["rdinv"] = rd.reshape(1, NBLK * 128)
        pm = np.zeros((NBLK * 128, G), dtype=fp8np)
        pm[np.arange(NPC), batch_new[lo:hi]] = 1.0
        pm = pm.reshape(NBLK, 128, G).transpose(1, 0, 2).reshape(128, NBLK * G)
        per_core[c]["pmat"] = np.ascontiguousarray(pm)

    struct = {"tiles": tiles, "calls": calls, "ttot": ttot, "itot": itot,
              "nt_block": nt_block}
    return struct, per_core, perm, dinv


def make_consts():
    ident = np.eye(128, dtype=np.float32)
    identb = np.eye(128, dtype=_np_dt(bf16))
    ones = np.ones((1, 128), dtype=np.float32)
    return {"ident": ident, "identb": identb, "ones": ones}


# ------------------------------------------------------------------ program --

def build(struct, timed_reps=None, has_bias=False):
    tiles = struct["tiles"]
    calls = struct["calls"]
    ttot = struct["ttot"]
    itot = struct["itot"]
    nt_block = struct["nt_block"]
    timed = timed_reps is not None

    nc = bacc.Bacc("TRN2", target_bir_lowering=False, debug=False,
                   num_devices=CORES, num_swdge_queues=QUEUES)

    xg = nc.dram_tensor("xg", [NPAIR, 2 * D], bf16, kind="ExternalInput")
    xloc = nc.dram_tensor("xloc", [NBLK * 128, D], bf16, kind="ExternalInput")
    idx = nc.dram_tensor("idx", [128, itot // 16], i16, kind="ExternalInput")
    bmat = nc.dram_tensor("bmat", [128, ttot * BLK], fp8, kind="ExternalInput")
    pmat = nc.dram_tensor("pmat", [128, NBLK * G], fp8, kind="ExternalInput")
    dinvc = nc.dram_tensor("dinvc", [128, NBLK], f32, kind="ExternalInput")
    dinv2c = nc.dram_tensor("dinv2c", [128, NBLK], f32, kind="ExternalInput")
    w1 = nc.dram_tensor("w1", [D, D], bf16, kind="ExternalInput")
    w2 = nc.dram_tensor("w2", [D, D], bf16, kind="ExternalInput")
    wh = nc.dram_tensor("wh", [D, DOUT], f32, kind="ExternalInput")
    bh = nc.dram_tensor("bh", [1, DOUT], f32, kind="ExternalInput")
    ident = nc.dram_tensor("ident", [128, 128], f32, kind="ExternalInput")
    identb = nc.dram_tensor("identb", [128, 128], bf16, kind="ExternalInput")
    ones = nc.dram_tensor("ones", [1, 128], f32, kind="ExternalInput")
    if has_bias:
        b1 = nc.dram_tensor("b1", [1, D], f32, kind="ExternalInput")
        b2 = nc.dram_tensor("b2", [1, D], f32, kind="ExternalInput")
        rdinv = nc.dram_tensor("rdinv", [1, NBLK * 128], f32,
                               kind="ExternalInput")
    out = nc.dram_tensor("out", [G, DOUT], f32, kind="ExternalOutput")

    with tile.TileContext(nc) as tc:
        with tc.tile_pool(name="const", bufs=1) as cp, \
             tc.tile_pool(name="gat", bufs=GATHER_BUFS) as gp, \
             tc.tile_pool(name="bt", bufs=B_BUFS) as bp, \
             tc.tile_pool(name="hs", bufs=4) as hp, \
             tc.tile_pool(name="agg", bufs=AGG_BUFS, space="PSUM") as aggp, \
             tc.tile_pool(name="hps", bufs=HPS_BUFS, space="PSUM") as hpsp, \
             tc.tile_pool(name="pl", bufs=1, space="PSUM") as plp, \
             tc.tile_pool(name="hd", bufs=1, space="PSUM") as hdp, \
             tc.tile_pool(name="dram", bufs=1, space="DRAM") as dp:

            idx_sb = cp.tile([128, itot // 16], i16)
            nc.sync.dma_start(idx_sb[:], idx[:])
            pmat_sb = cp.tile([128, NBLK * G], fp8)
            nc.sync.dma_start(pmat_sb[:], pmat[:])
            dinvc_sb = cp.tile([128, NBLK], f32)
            nc.sync.dma_start(dinvc_sb[:], dinvc[:])
            dinv2c_sb = cp.tile([128, NBLK], f32)
            nc.sync.dma_start(dinv2c_sb[:], dinv2c[:])
            w1_sb = cp.tile([D, D], bf16)
            nc.sync.dma_start(w1_sb[:], w1[:])
            w2_sb = cp.tile([D, D], bf16)
            nc.sync.dma_start(w2_sb[:], w2[:])
            wh_sb = cp.tile([D, DOUT], f32)
            nc.sync.dma_start(wh_sb[:], wh[:])
            bh_sb = cp.tile([1, DOUT], f32)
            nc.sync.dma_start(bh_sb[:], bh[:])
            id_sb = cp.tile([128, 128], f32)
            nc.sync.dma_start(id_sb[:], ident[:])
            idb_sb = cp.tile([128, 128], bf16)
            nc.sync.dma_start(idb_sb[:], identb[:])
            ones_sb = cp.tile([1, 128], f32)
            nc.sync.dma_start(ones_sb[:], ones[:])
            xslab = cp.tile([128, NBLK * D], bf16)
            nc.sync.dma_start(
                xslab[:].rearrange("p (a d) -> p a d", a=NBLK),
                xloc[:, :].rearrange("(a p) d -> p a d", p=128))
            slab2 = cp.tile([128, NBLK * D], bf16)
            if has_bias:
                b1_sb = cp.tile([1, D], f32)
                nc.sync.dma_start(b1_sb[:], b1[:])
                b2_sb = cp.tile([1, D], f32)
                nc.sync.dma_start(b2_sb[:], b2[:])
                rdinv_sb = cp.tile([1, NBLK * 128], f32)
                nc.sync.dma_start(rdinv_sb[:], rdinv[:])

            h1_bounce = dp.tile([NPC // 2, 2 * D], bf16)
            nrep = timed_reps if timed else 1
            h1_fulls = [dp.tile([NPAIR, 2 * D], bf16, addr_space="Shared",
                                name=f"h1_full_{r}") for r in range(nrep)]
            pool_ins = [dp.tile([G, D], f32, name=f"pool_in_{r}")
                        for r in range(nrep)]
            pool_outs = [dp.tile([G, D], f32, addr_space="Shared",
                                 name=f"pool_out_{r}") for r in range(nrep)]

            def do_layer(layer, table, slab_rd, w_sb, b_sb, scale_sb):
                st = {}             # b -> [agg_ps, ntiles_done]
                nfin = 0            # finished-block counter (pool start/stop)
                for ci, (t0, ct, p) in enumerate(calls):
                    ni = ct * 128
                    gat = gp.tile([128, CT_MAX, D], bf16, tag="gat")
                    nc.gpsimd.dma_gather(
                        gat[:, :ct, :],
                        table[:, p * D:(p + 1) * D],
                        idx_sb[:, t0 * 8:(t0 + ct) * 8],
                        ni, ni, D, elem_step=2 * D, single_packet=False,
                        queue_num=ci % QUEUES)
                    bseg = bp.tile([128, CT_MAX, BLK], fp8, tag="B")
                    nc.sync.dma_start(
                        bseg[:, :ct, :],
                        bmat[:, t0 * BLK:(t0 + ct) * BLK].rearrange(
                            "p (a c) -> p a c", a=ct))
                    for k in range(ct):
                        b, _p = tiles[t0 + k]
                        s = st.get(b)
                        if s is None:
                            agg_new = aggp.tile([128, BLK], f32, tag="agg",
                                                name="agg")
                            s = st[b] = [agg_new, 0]
                        nc.tensor.matmul(
                            s[0][:], lhsT=gat[:, k, :], rhs=bseg[:, k, :],
                            start=(s[1] == 0), stop=False,
                            skip_group_check=True)
                        s[1] += 1
                        if s[1] < nt_block[b]:
                            continue
                        # ---- finish block b
                        del st[b]
                        agg_ps = s[0]
                        w = BLK if b < NBLK - 1 else NPC - (NBLK - 1) * BLK
                        nc.tensor.matmul(
                            agg_ps[:, :w],
                            lhsT=slab_rd[:w, b * D:b * D + D],
                            rhs=idb_sb[:w, :w],
                            start=False, stop=True, skip_group_check=True)
                        aggt = hp.tile([128, BLK], bf16, tag="aggt")
                        nc.vector.tensor_copy(out=aggt[:], in_=agg_ps[:])
                        h_ps = hpsp.tile([128, 128], f32, tag="hps")
                        nc.tensor.matmul(
                            h_ps[:w, :], lhsT=aggt[:, :w], rhs=w_sb[:],
                            start=True, stop=not has_bias,
                            skip_group_check=True)
                        if has_bias:
                            nc.tensor.matmul(
                                h_ps[:w, :],
                                lhsT=rdinv_sb[:, b * 128:b * 128 + w],
                                rhs=b_sb[:], start=False, stop=True,
                                skip_group_check=True)
                        if layer == 1:
                            nc.scalar.activation(
                                slab2[:w, b * D:b * D + D], h_ps[:w, :],
                                mybir.ActivationFunctionType.Relu,
                                scale=scale_sb[:w, b:b + 1])
                        else:
                            h_sb = hp.tile([128, 128], bf16, tag="h")
                            nc.scalar.activation(
                                h_sb[:w, :], h_ps[:w, :],
                                mybir.ActivationFunctionType.Relu,
                                scale=scale_sb[:w, b:b + 1])
                            nc.tensor.matmul(
                                pool_ps[:],
                                lhsT=pmat_sb[:w, b * G:(b + 1) * G],
                                rhs=h_sb[:w, :],
                                start=(nfin == 0), stop=(nfin == NBLK - 1),
                                skip_group_check=True)
                        nfin += 1
                assert not st and nfin == NBLK

            def head(pl_sb):
                pt_ps = hdp.tile([D, G], f32, tag="hd")
                nc.tensor.matmul(pt_ps[:], lhsT=pl_sb[:], rhs=id_sb[:G, :G],
                                 start=True, stop=True, skip_group_check=True)
                pt_sb = hp.tile([D, G], f32, tag="pt")
                nc.vector.tensor_copy(out=pt_sb[:], in_=pt_ps[:])
                lg_ps = hdp.tile([G, DOUT], f32, tag="hd")
                nc.tensor.matmul(lg_ps[:], lhsT=pt_sb[:], rhs=wh_sb[:],
                                 start=True, stop=False)
                nc.tensor.matmul(lg_ps[:], lhsT=ones_sb[:, :G], rhs=bh_sb[:],
                                 start=False, stop=True)
                lg_sb = hp.tile([G, DOUT], f32, tag="lg")
                nc.vector.tensor_copy(out=lg_sb[:], in_=lg_ps[:])
                mx = hp.tile([G, 1], f32, tag="mx")
                nc.vector.reduce_max(mx[:], lg_sb[:], axis=mybir.AxisListType.X)
                nc.vector.tensor_scalar(out=lg_sb[:], in0=lg_sb[:],
                                        scalar1=mx[:], scalar2=None,
                                        op0=mybir.AluOpType.subtract)
                ex = hp.tile([G, DOUT], f32, tag="ex")
                nc.scalar.activation(ex[:], lg_sb[:],
                                     mybir.ActivationFunctionType.Exp)
                sm = hp.tile([G, 1], f32, tag="sm")
                nc.vector.reduce_sum(sm[:], ex[:], axis=mybir.AxisListType.X)
                ls = hp.tile([G, 1], f32, tag="ls")
                nc.scalar.activation(ls[:], sm[:],
                                     mybir.ActivationFunctionType.Ln)
                nc.vector.tensor_scalar(out=lg_sb[:], in0=lg_sb[:],
                                        scalar1=ls[:], scalar2=None,
                                        op0=mybir.AluOpType.subtract)
                nc.sync.dma_start(out[:, :], lg_sb[:])

            def whole(rep):
                do_layer(1, xg, xslab, w1_sb,
                         b1_sb if has_bias else None, dinv2c_sb)
                # slab2 -> h1_bounce ([NPC/2, 256] pair rows)
                nc.sync.dma_start(
                    h1_bounce[0:(NBLK - 1) * 64, :].rearrange(
                        "(a q) (e d) -> (q e) a d", q=64, e=2),
                    slab2[:, :(NBLK - 1) * D].rearrange(
                        "p (a d) -> p a d", a=NBLK - 1))
                wlast = NPC - (NBLK - 1) * BLK
                nc.sync.dma_start(
                    h1_bounce[(NBLK - 1) * 64:NPC // 2, :].rearrange(
                        "a (e d) -> (a e) d", e=2),
                    slab2[:wlast, (NBLK - 1) * D:NBLK * D])
                nc.gpsimd.collective_compute(
                    "AllGather", mybir.AluOpType.bypass,
                    replica_groups=[list(range(CORES))],
                    ins=[h1_bounce[:, :].opt()],
                    outs=[h1_fulls[rep][:, :].opt()])
                do_layer(2, h1_fulls[rep], slab2, w2_sb,
                         b2_sb if has_bias else None, dinvc_sb)
                pl_sb = hp.tile([G, D], f32, tag="pl")
                nc.scalar.activation(pl_sb[:], pool_ps[:],
                                     mybir.ActivationFunctionType.Copy)
                nc.sync.dma_start(pool_ins[rep][:, :], pl_sb[:])
                nc.gpsimd.collective_compute(
                    "AllReduce", mybir.AluOpType.add,
                    replica_groups=[list(range(CORES))],
                    ins=[pool_ins[rep][:, :].opt()],
                    outs=[pool_outs[rep][:, :].opt()])
                pl2_sb = hp.tile([G, D], f32, tag="pl2")
                nc.sync.dma_start(pl2_sb[:], pool_outs[rep][:, :])
                head(pl2_sb)

            for rep in range(nrep):
                pool_ps = plp.tile([G, D], f32, tag="pool")
                whole(rep)

    nc.compile()
    return nc


def make_in_maps(inputs, per_core, perm, dinv):
    import ml_dtypes
    consts = make_consts()
    x = np.asarray(inputs["x"], dtype=np.float32)
    xt = x * dinv[:, None]                      # dinv in old-id space
    x_perm = np.empty_like(xt)
    x_perm[perm] = xt
    xg16 = np.ascontiguousarray(x_perm.astype(ml_dtypes.bfloat16))
    has_bias = bool(np.any(inputs["b1"]) or np.any(inputs["b2"]))
    base = {
        "xg": xg16.reshape(NPAIR, 2 * D),
        "w1": np.asarray(inputs["W1"], dtype=ml_dtypes.bfloat16),
        "w2": np.asarray(inputs["W2"], dtype=ml_dtypes.bfloat16),
        "wh": np.asarray(inputs["Wh"], dtype=np.float32),
        "bh": np.asarray(inputs["bh"], dtype=np.float32).reshape(1, DOUT),
        **consts,
    }
    if has_bias:
        base["b1"] = np.asarray(inputs["b1"], dtype=np.float32).reshape(1, D)
        base["b2"] = np.asarray(inputs["b2"], dtype=np.float32).reshape(1, D)
    in_maps = []
    for c in range(CORES):
        m = dict(base)
        for k in ("idx", "bmat", "pmat", "dinvc", "dinv2c"):
            m[k] = per_core[c][k]
        if has_bias:
            m["rdinv"] = per_core[c]["rdinv"]
        xl = np.zeros((NBLK * 128, D), dtype=ml_dtypes.bfloat16)
        xl[:NPC] = xg16[c * NPC:(c + 1) * NPC]
        m["xloc"] = xl
        in_maps.append(m)
    return in_maps


def kernel(**inputs) -> np.ndarray:
    struct, per_core, perm, dinv = prep(inputs["edge_index"], inputs["batch"])
    has_bias = bool(np.any(inputs["b1"]) or np.any(inputs["b2"]))
    nc = build(struct, has_bias=has_bias)
    in_maps = make_in_maps(inputs, per_core, perm, dinv)
    from concourse.bass_utils import run_bass_kernel_spmd
    res = run_bass_kernel_spmd(nc, in_maps, core_ids=list(range(CORES)))
    return np.asarray(res.results[0]["out"], dtype=np.float32)


if __name__ == "__main__":
    import reference
    inputs = reference.setup_inputs()
    got = kernel(**{k: np.asarray(v) for k, v in inputs.items()})
    print(got[:2])
